# revision 5
# baseline (speedup 1.0000x reference)
"""Bahdanau attention with coverage — Trainium2 Bass kernel, 8-core data-parallel.

Math (per batch b):
  enc_p = X_b @ Wh + bh            X_b: (S=2048, H=1024)
  dec_p = d_b @ Ws + bs            (H,)
  cov_p = c_b @ Wc + bc            scalar
  score = tanh(enc_p + dec_p + cov_p) @ V + bv      (bv dropped: softmax shift-invariant)
  attn  = softmax(score)           over S
  cov_new = c_b + attn
  ctx   = attn^T @ X_b             (H,)

Strategy: batch-sharded over 8 cores (4 batches each). Compute enc_p in the
transposed layout enc_p^T (H on partitions, S on free) so the bias add + tanh
fuse into one ScalarE activation and the V-dot contracts over partitions.
X^T tiles come from PE-transposes of fp32r-rounded X. All big matmuls run in
fp32r (1 cycle/row at N=512, ~16x more accurate than bf16).

Sequence index mapping: s = 16*p + j  (p = partition, j = 0..15 "j-tile").
X j-tile j = X[j::16, :] (128 rows); this keeps every DMA 64B+ contiguous and
makes softmax/coverage layouts line up with DRAM order.
"""

import numpy as np

B, S, H = 32, 2048, 1024
NCORES = 8
BL = B // NCORES          # batches per core
NK = H // 128             # 8 h/k tiles
NJ = 16                   # j-tiles per batch (s = 16p + j)
NG = 4                    # s-blocks of 512 per batch
SBW = 512                 # s-block width

_CACHE = {}


def _build():
    import concourse.bacc as bacc
    import concourse.mybir as mybir
    import concourse.tile as tile
    from concourse.masks import make_identity

    f32 = mybir.dt.float32
    f32r = mybir.dt.float32r
    AF = mybir.ActivationFunctionType
    ALU = mybir.AluOpType

    nc = bacc.Bacc("TRN2", target_bir_lowering=False, debug=False,
                   num_devices=NCORES)

    eo = nc.dram_tensor("eo", (BL, S, H), f32, kind="ExternalInput").ap()
    dec = nc.dram_tensor("dec", (BL, H), f32, kind="ExternalInput").ap()
    cov = nc.dram_tensor("cov", (BL, S), f32, kind="ExternalInput").ap()
    wh_d = nc.dram_tensor("wh", (H, H), f32, kind="ExternalInput").ap()
    bh_d = nc.dram_tensor("bh", (H,), f32, kind="ExternalInput").ap()
    ws_d = nc.dram_tensor("ws", (H, H), f32, kind="ExternalInput").ap()
    bs_d = nc.dram_tensor("bs", (H,), f32, kind="ExternalInput").ap()
    v_d = nc.dram_tensor("v", (H, 1), f32, kind="ExternalInput").ap()
    wc_d = nc.dram_tensor("wc", (S, 1), f32, kind="ExternalInput").ap()
    bc_d = nc.dram_tensor("bc", (1,), f32, kind="ExternalInput").ap()

    ctx_o = nc.dram_tensor("ctx_o", (BL, H), f32, kind="ExternalOutput").ap()
    attn_o = nc.dram_tensor("attn_o", (BL, S, 1), f32, kind="ExternalOutput").ap()
    covn_o = nc.dram_tensor("covn_o", (BL, S), f32, kind="ExternalOutput").ap()

    with tile.TileContext(nc) as tc:
        with (
            tc.tile_pool(name="const", bufs=1) as cp,
            tc.tile_pool(name="wraw", bufs=2) as wraw,
            tc.tile_pool(name="whr", bufs=1) as whrp,
            tc.tile_pool(name="xr", bufs=1) as xrp,
            tc.tile_pool(name="x32", bufs=3) as x32p,
            tc.tile_pool(name="xt", bufs=2) as xtp,
            tc.tile_pool(name="tt", bufs=3) as tp,
            tc.tile_pool(name="u", bufs=2) as up,
            tc.tile_pool(name="sm", bufs=2) as smp,
            tc.tile_pool(name="ps_tr", bufs=2, space="PSUM") as ps_tr,
            tc.tile_pool(name="ps_e", bufs=2, space="PSUM") as ps_e,
            tc.tile_pool(name="ps_s", bufs=2, space="PSUM") as ps_s,
            tc.tile_pool(name="ps_c", bufs=2, space="PSUM") as ps_c,
        ):
            # ---------------- constants / setup ----------------
            ident_f = cp.tile([128, 128], f32)
            make_identity(nc, ident_f[:])
            ident_r = cp.tile([128, 128], f32r)
            nc.vector.tensor_copy(ident_r[:], ident_f[:])

            ones = cp.tile([128, 1], f32)
            nc.vector.memset(ones[:], 1.0)
            ones1 = cp.tile([1, 128], f32)
            nc.vector.memset(ones1[:], 1.0)

            # V -> v_sb[p, t] = V[128 t + p]
            v8 = cp.tile([8, 128], f32)
            nc.sync.dma_start(v8[:], v_d.rearrange("(t p) one -> t (p one)", t=8))
            v8r = cp.tile([8, 128], f32r)
            nc.vector.tensor_copy(v8r[:], v8[:])
            ps_v = ps_s.tile([128, 8], f32r, tag="small")
            nc.tensor.transpose(ps_v[:], v8r[:], ident_r[0:8, 0:8])
            v_sb = cp.tile([128, 8], f32)
            nc.scalar.copy(v_sb[:], ps_v[:])

            # (bh + bs) -> bhsT[p, t]
            bh8 = cp.tile([8, 128], f32)
            nc.sync.dma_start(bh8[:], bh_d.rearrange("(t p) -> t p", t=8))
            bs8 = cp.tile([8, 128], f32)
            nc.sync.dma_start(bs8[:], bs_d.rearrange("(t p) -> t p", t=8))
            bhs8 = cp.tile([8, 128], f32r)
            nc.vector.tensor_add(out=bhs8[:], in0=bh8[:], in1=bs8[:])
            ps_b = ps_s.tile([128, 8], f32r, tag="small")
            nc.tensor.transpose(ps_b[:], bhs8[:], ident_r[0:8, 0:8])
            bhsT = cp.tile([128, 8], f32)
            nc.scalar.copy(bhsT[:], ps_b[:])

            # decoder state -> dT[p, 4t + b] = d[b, 128 t + p]
            d8 = cp.tile([BL, H], f32)
            nc.sync.dma_start(d8[:], dec[:, :])
            d8r = cp.tile([BL, H], f32r)
            nc.vector.tensor_copy(d8r[:], d8[:])
            dT = cp.tile([128, NK * BL], f32)
            for t in range(NK):
                ps_d = ps_s.tile([128, BL], f32r, tag="small")
                nc.tensor.transpose(ps_d[:], d8r[0:BL, t * 128:(t + 1) * 128],
                                    ident_r[0:BL, 0:BL])
                nc.scalar.copy(dT[:, t * BL:(t + 1) * BL], ps_d[:])

            # Wh resident in SBUF, rounded to fp32r. wh_r[p, 1024 t + c] = Wh[128 t + p, c]
            wh_r = whrp.tile([128, NK * H], f32r)
            for t in range(NK):
                wt = wraw.tile([128, H], f32, tag="wraw")
                nc.sync.dma_start(wt[:], wh_d[t * 128:(t + 1) * 128, :])
                nc.vector.tensor_copy(wh_r[:, t * H:(t + 1) * H], wt[:])

            # coverage tiles (all batches up front; needed for cov_p and later
            # for coverage_new). c_sb[p, j] = c[16 p + j]
            wc_sb = cp.tile([128, NJ], f32)
            nc.sync.dma_start(wc_sb[:], wc_d.rearrange("(p j) one -> p (j one)", j=NJ))
            bc_sb = cp.tile([1, 1], f32)
            nc.sync.dma_start(bc_sb[:], bc_d.rearrange("(one two) -> one two", one=1))
            c_sbs = []
            cc = cp.tile([128, BL], f32)
            junk = cp.tile([128, NJ], f32)
            for b in range(BL):
                c_sb = smp.tile([128, NJ], f32, tag="c", bufs=BL)
                nc.sync.dma_start(c_sb[:], cov[b].rearrange("(p j) -> p j", j=NJ))
                c_sbs.append(c_sb)
                nc.vector.tensor_mul(out=junk[:], in0=c_sb[:], in1=wc_sb[:])
                nc.vector.reduce_sum(out=cc[:, b:b + 1], in_=junk[:],
                                     axis=mybir.AxisListType.X)
            ps_cp = ps_s.tile([1, BL], f32, tag="small")
            nc.tensor.matmul(ps_cp[:], ones[:], cc[:], start=True, stop=True)
            covp1 = cp.tile([1, BL], f32)
            nc.vector.tensor_scalar(out=covp1[:], in0=ps_cp[:], scalar1=bc_sb[:],
                                    scalar2=None, op0=ALU.add)
            ps_cb = ps_s.tile([128, BL], f32, tag="small")
            nc.tensor.matmul(ps_cb[:], ones1[:], covp1[:], start=True, stop=True)
            covp_bc = cp.tile([128, BL], f32)
            nc.scalar.copy(covp_bc[:], ps_cb[:])

            # dec_p^T + (bh+bs) + cov_p -> bias_sb[p, 4 k + b]
            # Ws streams through wraw; partial sums accumulate in SBUF.
            bias_sb = cp.tile([128, NK * BL], f32)
            dpacc = cp.tile([128, NK * BL], f32)
            for t in range(NK):
                wst = wraw.tile([128, H], f32, tag="wraw")
                nc.sync.dma_start(wst[:], ws_d[t * 128:(t + 1) * 128, :])
                for k in range(NK):
                    ps_dp = ps_s.tile([128, BL], f32, tag="small")
                    nc.tensor.matmul(
                        ps_dp[:], wst[:, k * 128:(k + 1) * 128],
                        dT[:, t * BL:(t + 1) * BL], start=True, stop=True)
                    if t == 0:
                        nc.vector.tensor_copy(dpacc[:, k * BL:(k + 1) * BL], ps_dp[:])
                    else:
                        nc.vector.tensor_add(
                            out=dpacc[:, k * BL:(k + 1) * BL],
                            in0=dpacc[:, k * BL:(k + 1) * BL], in1=ps_dp[:])
            for k in range(NK):
                nc.vector.tensor_scalar(
                    out=bias_sb[:, k * BL:(k + 1) * BL],
                    in0=dpacc[:, k * BL:(k + 1) * BL],
                    scalar1=bhsT[:, k:k + 1], scalar2=None, op0=ALU.add)
                nc.vector.tensor_add(
                    out=bias_sb[:, k * BL:(k + 1) * BL],
                    in0=bias_sb[:, k * BL:(k + 1) * BL], in1=covp_bc[:])

            # ---------------- main per-batch pipeline ----------------
            for b in range(BL):
                # load + round X_b; x_r[p, 1024 j + h] = X[16 p + j, h]
                x_r = xrp.tile([128, NJ * H], f32r, tag="xr")
                eo_b = eo[b].rearrange("(p j) h -> j p h", j=NJ)
                for j in range(NJ):
                    x32 = x32p.tile([128, H], f32, tag="x32")
                    nc.sync.dma_start(x32[:], eo_b[j])
                    nc.vector.tensor_copy(x_r[:, j * H:(j + 1) * H], x32[:])

                U = up.tile([128, S], f32, tag="u")

                for g in range(NG):
                    # transpose 4 j-tiles x 8 h-tiles -> xt (h on partitions)
                    xt = xtp.tile([128, NK * SBW], f32r, tag="xt")
                    for t in range(NK):
                        ps_t = ps_tr.tile([128, SBW], f32r, tag="tr")
                        for jj in range(4):
                            j = 4 * g + jj
                            nc.tensor.transpose(
                                ps_t[:, jj * 128:(jj + 1) * 128],
                                x_r[:, j * H + t * 128: j * H + (t + 1) * 128],
                                ident_r[:])
                        nc.scalar.copy(xt[:, t * SBW:(t + 1) * SBW], ps_t[:])

                    # enc_p^T tile (k on partitions, 512 s cols) + tanh + V-dot
                    for k in range(NK):
                        ps_mm = ps_e.tile([128, SBW], f32, tag="mm")
                        for t in range(NK):
                            nc.tensor.matmul(
                                ps_mm[:],
                                wh_r[:, t * H + k * 128: t * H + (k + 1) * 128],
                                xt[:, t * SBW:(t + 1) * SBW],
                                start=(t == 0), stop=(t == NK - 1))
                        tt = tp.tile([128, SBW], f32, tag="t")
                        nc.scalar.activation(tt[:], ps_mm[:], AF.Tanh,
                                             bias=bias_sb[:, k * BL + b: k * BL + b + 1],
                                             scale=1.0)
                        if k == 0:
                            nc.vector.tensor_scalar(
                                out=U[:, g * SBW:(g + 1) * SBW], in0=tt[:],
                                scalar1=v_sb[:, k:k + 1], scalar2=None, op0=ALU.mult)
                        else:
                            nc.vector.affine_then_add(
                                out=U[:, g * SBW:(g + 1) * SBW], in0=tt[:],
                                in1=U[:, g * SBW:(g + 1) * SBW],
                                scale=v_sb[:, k:k + 1], bias=0.0)

                # score^T: score_sb[p, j] = sum_part U[:, (j//4)*512 + (j%4)*128 + p]
                score_sb = smp.tile([128, NJ], f32, tag="score")
                for j in range(NJ):
                    ps_sc = ps_s.tile([128, 1], f32, tag="small")
                    uoff = (j // 4) * SBW + (j % 4) * 128
                    nc.tensor.matmul(ps_sc[:], U[:, uoff:uoff + 128], ones[:],
                                     start=True, stop=True)
                    nc.scalar.copy(score_sb[:, j:j + 1], ps_sc[:])

                # softmax over all 2048 (no max-subtraction; |score| <= ~16)
                esb = smp.tile([128, NJ], f32, tag="esb")
                acc = smp.tile([128, 1], f32, tag="acc")
                nc.scalar.activation(esb[:], score_sb[:], AF.Exp, accum_out=acc[:])
                ps_tot = ps_s.tile([1, 1], f32, tag="small")
                nc.tensor.matmul(ps_tot[:], acc[:], ones[:], start=True, stop=True)
                tot1 = smp.tile([1, 1], f32, tag="tot")
                nc.scalar.copy(tot1[:], ps_tot[:])
                rcp1 = smp.tile([1, 1], f32, tag="rcp")
                nc.vector.reciprocal(rcp1[:], tot1[:])
                ps_rb = ps_s.tile([128, 1], f32, tag="small")
                nc.tensor.matmul(ps_rb[:], ones1[:], rcp1[:], start=True, stop=True)
                rcp_sb = smp.tile([128, 1], f32, tag="rcpb")
                nc.scalar.copy(rcp_sb[:], ps_rb[:])

                attn_f = smp.tile([128, NJ], f32, tag="attnf")
                nc.vector.tensor_scalar_mul(attn_f[:], esb[:], rcp_sb[:])
                attn_r = smp.tile([128, NJ], f32r, tag="attnr")
                nc.vector.tensor_scalar_mul(attn_r[:], esb[:], rcp_sb[:])

                covn = smp.tile([128, NJ], f32, tag="covn")
                nc.vector.tensor_add(out=covn[:], in0=c_sbs[b][:], in1=attn_f[:])

                nc.sync.dma_start(attn_o[b].rearrange("(p j) one -> p (j one)", j=NJ),
                                  attn_f[:])
                nc.sync.dma_start(covn_o[b].rearrange("(p j) -> p j", j=NJ), covn[:])

                # context: ctx[h] = sum_j sum_p attn[p, j] * x_r[p, 1024 j + h]
                ctx_sb = smp.tile([1, H], f32, tag="ctx")
                for hh in range(2):
                    ps_cx = ps_c.tile([1, SBW], f32, tag="ctx")
                    for j in range(NJ):
                        nc.tensor.matmul(
                            ps_cx[:], attn_r[:, j:j + 1],
                            x_r[:, j * H + hh * SBW: j * H + hh * SBW + SBW],
                            start=(j == 0), stop=(j == NJ - 1))
                    nc.scalar.copy(ctx_sb[:, hh * SBW:(hh + 1) * SBW], ps_cx[:])
                nc.sync.dma_start(ctx_o[b:b + 1, :], ctx_sb[:])

    nc.compile()
    return nc


def _get_nc():
    if "nc" not in _CACHE:
        _CACHE["nc"] = _build()
    return _CACHE["nc"]


def run(trace=False, **inputs):
    from concourse.bass_utils import run_bass_kernel_spmd

    nc = _get_nc()
    f32 = np.float32

    def c(x):
        return np.ascontiguousarray(np.asarray(x, dtype=f32))

    in_maps = []
    for i in range(NCORES):
        sl = slice(i * BL, (i + 1) * BL)
        in_maps.append({
            "eo": c(inputs["encoder_output"][sl]),
            "dec": c(inputs["decoder_state"][sl]),
            "cov": c(inputs["coverage_vector"][sl]),
            "wh": c(inputs["Wh"]),
            "bh": c(inputs["bh"]),
            "ws": c(inputs["Ws"]),
            "bs": c(inputs["bs"]),
            "v": c(inputs["V"]),
            "wc": c(inputs["Wc"]),
            "bc": c(inputs["bc"]),
        })

    res = run_bass_kernel_spmd(nc, in_maps, list(range(NCORES)), trace=trace)

    ctx = np.concatenate([res.results[i]["ctx_o"] for i in range(NCORES)], axis=0)
    attn = np.concatenate([res.results[i]["attn_o"] for i in range(NCORES)], axis=0)
    covn = np.concatenate([res.results[i]["covn_o"] for i in range(NCORES)], axis=0)
    return (ctx, attn, covn), res


def kernel(**inputs):
    outs, _ = run(trace=False, **inputs)
    return outs
